# revision 55
# baseline (speedup 1.0000x reference)
"""Trainium2 Bass kernel for ChunkedLocalSelfAttention.

Module: x[B,C,H,W] -> qkv proj -> 8-head local-window attention (17x17
spatial window) -> out proj -> +residual -> 1x1 conv -> relu.
B,C,H,W = 4,256,48,48; N = 2304 tokens per image; head dim 32.

Sharding: 8 cores = 4 batch images x 2 query-row-halves (24 rows each).
Each core computes the full pipeline for its half-image: attention output
rows only depend on +-8 image rows, so cores need no communication; the
row halo is covered by computing k/v for a 32-row band.

On-core design (scores kept TRANSPOSED: keys on partitions, queries free).
Attention is blocked in COLUMN BANDS of 16: queries of band b are the 24
rows x 16 cols [16b, 16b+16); its keys live in ext cols (+-8 halo,
clipped) x 32 rows, chunked into 8 chunks of 4 rows (<=128 keys). This
streams ~29% fewer score columns than full-width 128-token chunks since
the column halo is 24-32 wide instead of 48.
  - qk projection: qkT [512, 2304] = WqkT.T @ xT, bf16 (bias on DVE);
    score lhsT/rhs slices use strided row x col access patterns.
  - v is re-laid per band chunk in [token, channel] order (partitions must
    match the band token order of the score rows); v bias is folded into
    the out-projection bias on the host (softmax weights sum to 1).
  - per (band, head-pair g, chunk): scoresT via row-packed K=32 matmuls,
    exp on ScalarE (scale fused), binary window mask multiply on VectorE,
    PV+sums accumulate via col-packed matmuls (ones lhsT strips replicate
    each head's sum onto the 32 partitions under its pv rows). The first
    PV matmul opens the bank with start=True (no zeroing matmul).
  - on = pp * shifted recip(sums) -> bf16; out proj reads the per-group
    `on` tiles directly with zero-padded weight rows (no compaction),
    +residual from xT with the folded bias, 1x1 conv, bias+relu on
    ScalarE, bf16 band-major output (host un-permutes).
"""

import sys

for _p in ("/opt/trn_rl_repo",):
    if _p not in sys.path:
        sys.path.insert(0, _p)

import math

import ml_dtypes
import numpy as np

B, C, H, W = 4, 256, 48, 48
N = H * W
HEADS, HD, HALF = 8, 32, 8
NCORES = 8
ROWS_HALF = H // 2          # 24 query rows per core
NQ = ROWS_HALF * W          # 1152 queries per core
BAND_ROWS = 32              # k/v row band per core (24 + 8 halo)
QT = 384                    # queries per band tile (24 rows x 16 cols)
NB = 3                      # column bands

SCALE = 1.0 / math.sqrt(HD)

# per band: first ext col, ext width
BANDC = [(0, 24), (8, 32), (24, 24)]
# per band: key chunk row ranges (chunk keys = rows x ext cols <= 128):
# edge bands (ew=24) use 5-row chunks (120 keys), center (ew=32) 4-row
_CH_EDGE = [(5 * j, min(32, 5 * j + 5)) for j in range(7)]
_CH_CENTER = [(4 * j, 4 * j + 4) for j in range(8)]
CH = [_CH_EDGE, _CH_CENTER, _CH_EDGE]
# query row window per chunk: [r0-8, r1+8) clipped to [0, 24)
def _win(r0, r1):
    return (max(0, r0 - 8), min(24, r1 + 8))
# packed mask column offsets: per band, per chunk
_off = 0
MOFF = []
for _b in range(NB):
    row = []
    for (_r0, _r1) in CH[_b]:
        row.append(_off)
        _rlo, _rhi = _win(_r0, _r1)
        _off += (_rhi - _rlo) * 16
    MOFF.append(row)
MTOT = _off

bf16 = ml_dtypes.bfloat16

_PROG = None


def _build_program():
    import concourse.bass as bass
    import concourse.mybir as mybir
    import concourse.tile as tile
    from concourse import bacc

    f32 = mybir.dt.float32
    bft = mybir.dt.bfloat16
    AF = mybir.ActivationFunctionType
    OP = mybir.AluOpType

    nc = bacc.Bacc(
        "TRN2", target_bir_lowering=False, debug=False, num_devices=NCORES
    )

    def din(name, shape, dt=bft):
        return nc.dram_tensor(name, shape, dt, kind="ExternalInput").ap()

    xt_d = din("xT", [128, 2, 1536])
    wqk_d = din("wqk", [128, 2, 2 * C])
    wv_d = din("wv", [128, 2, C])
    wo_d = din("wopad", [128, 4 * C])
    wc_d = din("wc", [128, 2, C])
    bias_d = din("biases", [128, 8], f32)
    mask_d = din("masks", [128, MTOT])
    out_d = nc.dram_tensor("out", [C, NB, QT], bft, kind="ExternalOutput").ap()

    # SPMD trick: one program must serve both row-halves. The host ships
    # half-1 images VERTICALLY FLIPPED (attention is equivariant under a
    # row flip; the window test is |dh|<=8), so every core sees half-0
    # geometry: query rows [0, 24), key band rows [0, 32).

    with tile.TileContext(nc) as tc:
        import contextlib

        ctx = contextlib.ExitStack()
        with ctx:
            cpool = ctx.enter_context(tc.tile_pool(name="const", bufs=1))
            qkpool = ctx.enter_context(tc.tile_pool(name="qk", bufs=1))
            vpool = ctx.enter_context(tc.tile_pool(name="v", bufs=1))
            epool = ctx.enter_context(tc.tile_pool(name="exp", bufs=4))
            apool = ctx.enter_context(tc.tile_pool(name="attn", bufs=4))
            rpool = ctx.enter_context(tc.tile_pool(name="recip", bufs=3))
            opool = ctx.enter_context(tc.tile_pool(name="outb", bufs=3))
            psA = ctx.enter_context(
                tc.tile_pool(name="psA", bufs=2, space="PSUM")
            )
            psB = ctx.enter_context(
                tc.tile_pool(name="psB", bufs=2, space="PSUM")
            )

            # ---- constants / inputs to SBUF (issue order = need order) ----
            # x rows 32-48 are never read (q uses [0,1152), k/v the 32-row
            # band [0,1536)), so only 1536 tokens are shipped.
            NX = 1536
            wqk = cpool.tile([128, 2, 2 * C], bft, tag="wqk")
            nc.sync.dma_start(wqk[:], wqk_d[:])
            xt = cpool.tile([128, 2, NX], bft, tag="xt")
            # 4 pieces so the first qk matmuls start ~3us earlier
            for pc in range(4):
                nc.sync.dma_start(
                    xt[:, :, 384 * pc : 384 * pc + 384],
                    xt_d[:, :, 384 * pc : 384 * pc + 384],
                )
            bias = cpool.tile([128, 8], f32, tag="bias")
            nc.sync.dma_start(bias[:], bias_d[:])
            msk = cpool.tile([128, MTOT], bft, tag="msk")
            nc.sync.dma_start(msk[:, 0 : MOFF[1][0]], mask_d[:, 0 : MOFF[1][0]])
            wv = cpool.tile([128, 2, C], bft, tag="wv")
            nc.sync.dma_start(wv[:], wv_d[:])
            nc.sync.dma_start(msk[:, MOFF[1][0] : MTOT], mask_d[:, MOFF[1][0] : MTOT])
            wo = cpool.tile([128, 4 * C], bft, tag="wo")
            nc.sync.dma_start(wo[:], wo_d[:])
            wc = cpool.tile([128, 2, C], bft, tag="wc")
            nc.sync.dma_start(wc[:], wc_d[:])
            zrow = cpool.tile([1, 512], bft, tag="zrow")
            nc.vector.memset(zrow[:], 0.0)

            # PE p-state warm-up: ~2.5us of junk matmuls so the real
            # projections hit the 3us-continuous-busy full-speed state
            # right as their inputs land (the ramp otherwise doubles the
            # first ~3us of matmul time)
            for _w in range(6):
                wps = psA.tile([128, 1024], f32, tag="sc", name="sc")
                nc.tensor.matmul(
                    wps[:, 0:512],
                    lhsT=zrow[:, 0:128],
                    rhs=zrow[:, 0:512],
                    start=True,
                    stop=True,
                )

            # ---- phase 1: qk projection  qkT[512, N] bf16 ----
            # q needed for tokens [0, 1152) only; k for the band [0, 1536)
            qk = [qkpool.tile([128, 1536], bft, tag=f"qk{i}", name=f"qk{i}") for i in range(4)]
            NT_Q = [(0, 384), (384, 384), (768, 384)]
            NT_K = [(0, 512), (512, 512), (1024, 512)]

            def qk_proj(qc):
                for n0, nw in (NT_Q if qc < 2 else NT_K):
                    ps = psB.tile([128, 512], f32, tag="ps", name="ps")
                    for cc in range(2):
                        nc.tensor.matmul(
                            ps[:, :nw],
                            lhsT=wqk[:, cc, 128 * qc : 128 * qc + 128],
                            rhs=xt[:, cc, n0 : n0 + nw],
                            start=(cc == 0),
                            stop=(cc == 1),
                        )
                    if qc < 2:
                        # q bias on DVE (fused with the bf16 cast)
                        nc.vector.tensor_scalar_add(
                            qk[qc][:, n0 : n0 + nw], ps[:, :nw], bias[:, qc : qc + 1]
                        )
                    else:
                        # the K bias shifts every key's logit by a constant
                        # per query, which softmax cancels exactly — drop
                        # it; the cast runs on the (early-idle) Act engine
                        nc.scalar.copy(qk[qc][:, n0 : n0 + nw], ps[:, :nw])

            # q reordered into band-major token order (matmul RHS APs must
            # have a single free dimension, so the band slices have to be
            # contiguous): qkb[ti][:, 384b + (r*16+c)] = q token (r, 16b+c)
            qkb = [qkpool.tile([128, NB * QT], bft, tag=f"qkb{i}", name=f"qkb{i}") for i in range(2)]

            def q_reorder(ti):
                src = qk[ti][:, 0:NQ].rearrange(
                    "p (r b c) -> p b r c", b=NB, c=16
                )
                nc.vector.tensor_copy(
                    qkb[ti][:].rearrange("p (b q) -> p b q", b=NB), src
                )

            # k and x re-laid in band-ext token order (matmul weight APs
            # must be a single free dimension too): 32 rows x ew cols per
            # band, bands packed at KOFF
            KOFF = [0, 768, 1792]
            KTOT = 2560
            kb = [qkpool.tile([128, KTOT], bft, tag=f"kb{i}", name=f"kb{i}") for i in range(2)]
            xb = cpool.tile([128, 2, KTOT], bft, tag="xb")

            def k_reorder(ti, bb):
                c0, ew = BANDC[bb]
                src = qk[2 + ti][:, :].rearrange("p (r w) -> p r w", w=W)[
                    :, :, c0 : c0 + ew
                ]
                dst = kb[ti][:, KOFF[bb] : KOFF[bb] + 32 * ew].rearrange(
                    "p (r c) -> p r c", c=ew
                )
                nc.vector.tensor_copy(dst, src)

            def x_reorder(cc, bb):
                c0, ew = BANDC[bb]
                src = xt[:, cc, 0 : 32 * W].rearrange("p (r w) -> p r w", w=W)[
                    :, :, c0 : c0 + ew
                ]
                dst = xb[:, cc, KOFF[bb] : KOFF[bb] + 32 * ew].rearrange(
                    "p (r c) -> p r c", c=ew
                )
                nc.vector.tensor_copy(dst, src)

            # v band chunk tiles, token-major partitions in band order
            # (chunk rows x extW); layout per tile: head h cols [64h,
            # 64h+32) = v_h, [64h+32, 64h+64) = 1.0
            vt = [
                [vpool.tile([128, 8 * 64], bft, tag=f"v{bb}_{j}", name=f"v{bb}_{j}")
                 for j in range(len(CH[bb]))]
                for bb in range(NB)
            ]

            def v_proj(bb, j):
                c0, ew = BANDC[bb]
                r0, r1 = CH[bb][j]
                m = (r1 - r0) * ew
                ko = KOFF[bb] + r0 * ew
                ps = psB.tile([128, 512], f32, tag="ps", name="ps")
                for cc in range(2):
                    nc.tensor.matmul(
                        ps[0:m, :C],
                        lhsT=xb[:, cc, ko : ko + m],
                        rhs=wv[:, cc, :],
                        start=(cc == 0),
                        stop=(cc == 1),
                    )
                va = vt[bb][j][:].rearrange("p (h two v) -> p h two v", two=2, v=32)
                cp = nc.vector if (bb + j) % 2 else nc.scalar
                if cp is nc.vector:
                    nc.vector.tensor_copy(
                        va[0:m, :, 0, :],
                        ps[0:m, :C].rearrange("p (h v) -> p h v", v=32),
                    )
                else:
                    nc.scalar.copy(
                        va[0:m, :, 0, :],
                        ps[0:m, :C].rearrange("p (h v) -> p h v", v=32),
                    )
                nc.gpsimd.memset(va[0:m, :, 1, :], 1.0)

            # Only band-0 / heads-0-3 inputs are prepared up front so
            # attention starts early; the rest is emitted into the PE
            # slack between group-pairs (see the hook schedule below).
            qk_proj(0)
            qk_proj(2)
            q_reorder(0)
            k_reorder(0, 0)
            for cc in range(2):
                x_reorder(cc, 0)
            v_proj(0, 0)
            v_proj(0, 1)

            def L(*fns):
                def emit():
                    for f in fns:
                        f()
                return emit

            # ---- phase 3: attention, per column band ----
            # Head-pair groups are processed in interleaved PAIRS (two
            # independent chunk pipelines) so each engine always has a
            # ready instruction from the other stream — single-stream
            # chunk chains leave 300-600ns handoff bubbles per chunk.
            res = [cpool.tile([128, NB * QT], bft, tag=f"res{t}", name=f"res{t}") for t in range(2)]

            def emit_scores(bb, g, ck):
                c0, ew = BANDC[bb]
                r0, r1 = CH[bb][ck]
                m = (r1 - r0) * ew
                ko = KOFF[bb] + r0 * ew
                rlo, rhi = _win(r0, r1)
                a, b = 16 * rlo, 16 * rhi
                qw_ = b - a
                sc = psA.tile([128, 1024], f32, tag="sc", name="sc")
                for hh in range(2):
                    h = 2 * g + hh
                    ti, krow = h // 4, 32 * (h % 4)
                    nc.tensor.matmul(
                        sc[0:m, 512 * hh + a : 512 * hh + b],
                        lhsT=kb[ti][krow : krow + 32, ko : ko + m],
                        rhs=qkb[ti][krow : krow + 32, QT * bb + a : QT * bb + b],
                        start=True,
                        stop=True,
                        tile_position=(krow, 0),
                    )
                ex = epool.tile([128, 2 * QT], bft, tag="ex", name="ex")
                sc_v = sc[0:m].rearrange("p (h q) -> p h q", q=512)[:, :, a:b]
                ex_v = ex[0:m].rearrange("p (h q) -> p h q", q=QT)[:, :, a:b]
                nc.scalar.activation(ex_v, sc_v, AF.Exp, scale=SCALE)
                ma = apool.tile([128, 2 * QT], bft, tag="ma", name="ma")
                ma_v = ma[0:m].rearrange("p (h q) -> p h q", q=QT)[:, :, a:b]
                mk = msk[0:m, MOFF[bb][ck] : MOFF[bb][ck] + qw_]
                nc.vector.tensor_mul(
                    ma_v, ex_v, mk[:, None, :].broadcast_to([m, 2, qw_])
                )
                return ma

            def emit_pv(bb, g, ck, ma, pp):
                c0, ew = BANDC[bb]
                r0, r1 = CH[bb][ck]
                m = (r1 - r0) * ew
                rlo, rhi = _win(r0, r1)
                a, b = 16 * rlo, 16 * rhi
                vi = vt[bb][ck]
                for hh in range(2):
                    h = 2 * g + hh
                    nc.tensor.matmul(
                        pp[64 * hh : 64 * hh + 64, a:b],
                        lhsT=vi[0:m, 64 * h : 64 * h + 64],
                        rhs=ma[0:m, QT * hh + a : QT * hh + b],
                        start=False,
                        stop=(ck == len(CH[bb]) - 1 and hh == 1),
                        skip_group_check=True,
                        tile_position=(0, 64 * hh),
                    )

            def emit_chunk(bb, g, ck, pp):
                emit_pv(bb, g, ck, emit_scores(bb, g, ck), pp)

            def emit_normalize(bb, g, pp):
                # rows of pp: 0-31 pv_a, 32-63 sums_a, 64-95 pv_b, 96-127 sums_b
                rc = rpool.tile([128, QT], f32, tag="rc", name="rc")
                nc.vector.reciprocal(rc[:], pp[:, 0:QT])
                on = rpool.tile([128, QT], bft, tag=f"on{g}", name=f"on{g}", bufs=2)
                if bb == NB - 1:
                    # shift-free normalize on the latency-critical last
                    # band: 32-partition base-offset multiplies (legal for
                    # PSUM x SBUF operands); rows 32-63 are read by the
                    # out proj against zero weights, so memset them.
                    nc.gpsimd.memset(on[32:64, :], 0.0)
                    nc.vector.tensor_mul(on[0:32, :], pp[0:32, 0:QT], rc[32:64, :])
                    nc.vector.tensor_mul(on[64:96, :], pp[64:96, 0:QT], rc[96:128, :])
                else:
                    # shift recip(sums) down 32 partitions onto pv lanes
                    rcs = rpool.tile([128, QT], f32, tag="rcs", name="rcs")
                    nc.sync.dma_start(rcs[0:96, :], rc[32:128, :])
                    nc.vector.tensor_mul(on[0:96, :], pp[0:96, 0:QT], rcs[0:96, :])
                return on

            # ---- projections for a band's columns (emitted after the
            # NEXT band's first pair so the PE stream hides them; the on
            # tiles rotate on a 2-ring so they stay live) ----
            def make_proj(bb, ons):
                def emit():
                    n0 = QT * bb
                    for oc in range(2):
                        ps = psB.tile([128, 512], f32, tag="ps", name="ps")
                        for g in range(4):
                            # out proj reads on tiles directly: rows 0-31 /
                            # 64-95 = normalized pv of heads 2g / 2g+1;
                            # wopad rows 32-63 are zero.
                            nc.tensor.matmul(
                                ps[:, :QT],
                                lhsT=wo[0:96, C * g + 128 * oc : C * g + 128 * oc + 128],
                                rhs=ons[g][0:96, :],
                                start=(g == 0),
                                stop=(g == 3),
                            )
                        xv = xt[:, oc, 0:NQ].rearrange(
                            "p (r w) -> p r w", w=W
                        )[:, :, 16 * bb : 16 * bb + 16]
                        nc.vector.scalar_tensor_tensor(
                            res[oc][:, n0 : n0 + QT].rearrange(
                                "p (r w) -> p r w", w=16
                            ),
                            ps[:, :QT].rearrange("p (r w) -> p r w", w=16),
                            bias[:, 6 + oc : 7 + oc],
                            xv,
                            OP.add,
                            OP.add,
                        )
                    for oc in range(2):
                        ps = psB.tile([128, 512], f32, tag="ps", name="ps")
                        for cc in range(2):
                            nc.tensor.matmul(
                                ps[:, :QT],
                                lhsT=wc[:, cc, 128 * oc : 128 * oc + 128],
                                rhs=res[cc][:, n0 : n0 + QT],
                                start=(cc == 0),
                                stop=(cc == 1),
                            )
                        ob = opool.tile([128, QT], bft, tag="ob", name="ob")
                        nc.scalar.activation(
                            ob[:],
                            ps[:, :QT],
                            AF.Relu,
                            bias=bias[:, 4 + oc : 5 + oc],
                        )
                        nc.sync.dma_start(
                            out_d[128 * oc : 128 * oc + 128, bb, :], ob[:]
                        )
                return emit

            # hook schedule: prep work spread finely into the PE/DVE slack
            # while the current pair's exp/mask stream runs — coarse
            # injections cause local engine bursts that starve Act
            after_pair = {}
            mid_hooks = {
                # band 0 remaining v tiles
                (0, 0, 0): [L(lambda: v_proj(0, 2), lambda: v_proj(0, 3))],
                (0, 0, 1): [L(*[lambda j=j: v_proj(0, j) for j in range(4, len(CH[0]))])],
                # heads 4-7 prep, needed by (0, 2)
                (0, 0, 2): [lambda: qk_proj(1)],
                (0, 0, 3): [lambda: qk_proj(3)],
                (0, 0, 4): [L(lambda: q_reorder(1), lambda: k_reorder(1, 0))],
                # band 1 prep, needed by (1, 0)
                (0, 2, 1): [L(lambda: k_reorder(0, 1), lambda: k_reorder(1, 1))],
                (0, 2, 2): [L(lambda: x_reorder(0, 1), lambda: x_reorder(1, 1))],
                (0, 2, 3): [L(*[lambda j=j: v_proj(1, j) for j in (0, 1, 2)])],
                (0, 2, 4): [L(*[lambda j=j: v_proj(1, j) for j in (3, 4)])],
                (0, 2, 5): [L(*[lambda j=j: v_proj(1, j) for j in range(5, len(CH[1]))])],
                # band 2 prep, needed by (2, 0)
                (1, 2, 1): [L(lambda: k_reorder(0, 2), lambda: k_reorder(1, 2))],
                (1, 2, 2): [L(lambda: x_reorder(0, 2), lambda: x_reorder(1, 2))],
                (1, 2, 3): [L(*[lambda j=j: v_proj(2, j) for j in (0, 1, 2)])],
                (1, 2, 4): [L(*[lambda j=j: v_proj(2, j) for j in (3, 4)])],
                (1, 2, 5): [L(*[lambda j=j: v_proj(2, j) for j in range(5, len(CH[2]))])],
            }

            # For the last band the out-proj accumulation is split: g0/g1
            # matmuls are emitted as soon as pair 0 normalizes (hidden
            # under pair 1's chunk stream), so the tail only runs g2/g3.
            last_ps = {}

            def proj_start_last(bb, ons):
                for oc in range(2):
                    ps = psB.tile([128, 512], f32, tag="ps", name="ps")
                    for g in (0, 1):
                        nc.tensor.matmul(
                            ps[:, :QT],
                            lhsT=wo[0:96, C * g + 128 * oc : C * g + 128 * oc + 128],
                            rhs=ons[g][0:96, :],
                            start=(g == 0),
                            stop=False,
                            skip_group_check=True,
                        )
                    last_ps[oc] = ps

            def proj_finish_last(bb, ons):
                n0 = QT * bb
                for oc in range(2):
                    ps = last_ps[oc]
                    for g in (2, 3):
                        nc.tensor.matmul(
                            ps[:, :QT],
                            lhsT=wo[0:96, C * g + 128 * oc : C * g + 128 * oc + 128],
                            rhs=ons[g][0:96, :],
                            start=False,
                            stop=(g == 3),
                            skip_group_check=True,
                        )
                    xv = xt[:, oc, 0:NQ].rearrange("p (r w) -> p r w", w=W)[
                        :, :, 16 * bb : 16 * bb + 16
                    ]
                    nc.vector.scalar_tensor_tensor(
                        res[oc][:, n0 : n0 + QT].rearrange("p (r w) -> p r w", w=16),
                        ps[:, :QT].rearrange("p (r w) -> p r w", w=16),
                        bias[:, 6 + oc : 7 + oc],
                        xv,
                        OP.add,
                        OP.add,
                    )
                for oc in range(2):
                    ps = psB.tile([128, 512], f32, tag="ps", name="ps")
                    for cc in range(2):
                        nc.tensor.matmul(
                            ps[:, :QT],
                            lhsT=wc[:, cc, 128 * oc : 128 * oc + 128],
                            rhs=res[cc][:, n0 : n0 + QT],
                            start=(cc == 0),
                            stop=(cc == 1),
                        )
                    ob = opool.tile([128, QT], bft, tag="ob", name="ob")
                    nc.scalar.activation(
                        ob[:], ps[:, :QT], AF.Relu, bias=bias[:, 4 + oc : 5 + oc]
                    )
                    nc.sync.dma_start(
                        out_d[128 * oc : 128 * oc + 128, bb, :], ob[:]
                    )

            proj = {}
            for bb in range(NB):
                ons = []
                for gp in (0, 2):
                    # The first chunk's scores/exp/mask are emitted BEFORE
                    # the pp zeroing matmuls: the zero-matmuls wait for the
                    # previous pair's normalize to free a pp ring slot, and
                    # the in-order PE queue would stall the new pair's
                    # whole score stream behind them.
                    mas = {g: emit_scores(bb, g, 0) for g in (gp, gp + 1)}
                    # pair tiles pp: rows = [pv_h|sums_h|pv_h'|sums_h'] for
                    # heads (2g, 2g+1); zeroed via an explicit start=True
                    # matmul (start=False on a cleared bank does NOT
                    # reliably SET on first write).
                    pps = {}
                    for g in (gp, gp + 1):
                        pp = psB.tile([128, 512], f32, tag="pp", name="pp", bufs=2)
                        nc.tensor.matmul(
                            pp[:, 0:QT],
                            lhsT=zrow[:, 0:128],
                            rhs=zrow[:, 0:QT],
                            start=True,
                            stop=False,
                            skip_group_check=True,
                        )
                        pps[g] = pp
                    for g in (gp, gp + 1):
                        emit_pv(bb, g, 0, mas[g], pps[g])
                    for fn in mid_hooks.get((bb, gp, 0), []):
                        fn()
                    for ck in range(1, len(CH[bb])):
                        for g in (gp, gp + 1):
                            emit_chunk(bb, g, ck, pps[g])
                        for fn in mid_hooks.get((bb, gp, ck), []):
                            fn()
                    for g in (gp, gp + 1):
                        ons.append(emit_normalize(bb, g, pps[g]))
                    for fn in after_pair.get((bb, gp), []):
                        fn()
                    if gp == 0 and bb > 0:
                        proj[bb - 1]()
                    if bb == NB - 1 and gp == 0:
                        proj_start_last(bb, ons)
                if bb < NB - 1:
                    proj[bb] = make_proj(bb, ons)
            proj_finish_last(NB - 1, ons)

    nc.compile()
    return nc


def _get_program():
    global _PROG
    if _PROG is None:
        _PROG = _build_program()
    return _PROG


def _prep_core_inputs(core, x, in_proj_w, in_proj_b, out_w, out_b, conv_w, conv_b):
    b, half = core // 2, core % 2
    ximg = x[b].reshape(C, H, W)
    if half == 1:
        ximg = ximg[:, ::-1, :]  # row-flip: half-1 becomes half-0 geometry
    xt = ximg.reshape(2, 128, N).transpose(1, 0, 2)

    wqk = in_proj_w[: 2 * C].T.reshape(2, 128, 2 * C).transpose(1, 0, 2)
    wv = in_proj_w[2 * C :].T.reshape(2, 128, C).transpose(1, 0, 2)
    wc = conv_w.T.reshape(2, 128, C).transpose(1, 0, 2)
    woT = out_w.T  # [in_ch, out_ch]
    wopad = np.zeros((128, 4 * C), np.float32)
    for g in range(4):
        wopad[0:32, C * g : C * g + C] = woT[64 * g : 64 * g + 32]
        wopad[64:96, C * g : C * g + C] = woT[64 * g + 32 : 64 * g + 64]

    # v bias folds through attention (softmax weights sum to 1) into the
    # out-proj bias: res = o@WoT + (out_b + bv@WoT) + x
    hostbias = out_b + in_proj_b[2 * C :] @ woT
    biases = np.zeros((128, 8), np.float32)
    biases[:, 0:4] = in_proj_b[: 2 * C].reshape(4, 128).T
    biases[:, 4:6] = conv_b.reshape(2, 128).T
    biases[:, 6:8] = hostbias.reshape(2, 128).T

    return {
        "xT": np.ascontiguousarray(xt[:, :, 0:1536]).astype(bf16),
        "wqk": np.ascontiguousarray(wqk).astype(bf16),
        "wv": np.ascontiguousarray(wv).astype(bf16),
        "wopad": wopad.astype(bf16),
        "wc": np.ascontiguousarray(wc).astype(bf16),
        "biases": biases,
        "masks": _masks(),
    }


_MASK_CACHE = {}


def _masks() -> np.ndarray:
    """[128, MTOT] binary window masks, packed per (band, chunk).

    Keys of chunk (band, j): rows [4j, 4j+4) x ext cols [c0, c0+ew),
    partition index (r-4j)*ew + (c-c0). Queries: rows [rlo, rhi) x band
    cols [16b, 16b+16), packed column index (r-rlo)*16 + (c-16b).
    """
    if "m" in _MASK_CACHE:
        return _MASK_CACHE["m"]
    out = np.zeros((128, MTOT), np.float32)
    for bb in range(NB):
        c0, ew = BANDC[bb]
        for j, (r0, r1) in enumerate(CH[bb]):
            rlo, rhi = _win(r0, r1)
            nk = (r1 - r0) * ew
            kr = r0 + np.arange(nk) // ew
            kc = c0 + np.arange(nk) % ew
            qr = rlo + np.arange((rhi - rlo) * 16) // 16
            qc = 16 * bb + np.arange((rhi - rlo) * 16) % 16
            m = (np.abs(kr[:, None] - qr[None, :]) <= HALF) & (
                np.abs(kc[:, None] - qc[None, :]) <= HALF
            )
            off = MOFF[bb][j]
            out[0:nk, off : off + (rhi - rlo) * 16] = m
    res = out.astype(bf16)
    _MASK_CACHE["m"] = res
    return res


def kernel(**inputs):
    from concourse.bass_utils import run_bass_kernel_spmd

    args = {k: np.asarray(v) for k, v in inputs.items()}
    nc = _get_program()
    in_maps = [
        _prep_core_inputs(core, **args) for core in range(NCORES)
    ]
    res = run_bass_kernel_spmd(nc, in_maps, core_ids=list(range(NCORES)))
    out = np.zeros((B, C, H, W), np.float32)
    for core in range(NCORES):
        b, half = core // 2, core % 2
        o = res.results[core]["out"].astype(np.float32)
        # band-major [C, 3, 24*16] -> [C, 24, 48]
        o = o.reshape(C, NB, ROWS_HALF, 16).transpose(0, 2, 1, 3).reshape(
            C, ROWS_HALF, W
        )
        if half == 1:
            o = o[:, ::-1, :]  # undo the row flip
            out[b][:, ROWS_HALF:, :] = o
        else:
            out[b][:, :ROWS_HALF, :] = o
    return out


# revision 56
# speedup vs baseline: 1.0572x; 1.0572x over previous
"""Trainium2 Bass kernel for ChunkedLocalSelfAttention.

Module: x[B,C,H,W] -> qkv proj -> 8-head local-window attention (17x17
spatial window) -> out proj -> +residual -> 1x1 conv -> relu.
B,C,H,W = 4,256,48,48; N = 2304 tokens per image; head dim 32.

Sharding: 8 cores = 4 batch images x 2 query-row-halves (24 rows each).
Each core computes the full pipeline for its half-image: attention output
rows only depend on +-8 image rows, so cores need no communication; the
row halo is covered by computing k/v for a 32-row band.

On-core design (scores kept TRANSPOSED: keys on partitions, queries free).
Attention is blocked in COLUMN BANDS of 16: queries of band b are the 24
rows x 16 cols [16b, 16b+16); its keys live in ext cols (+-8 halo,
clipped) x 32 rows, chunked into 8 chunks of 4 rows (<=128 keys). This
streams ~29% fewer score columns than full-width 128-token chunks since
the column halo is 24-32 wide instead of 48.
  - qk projection: qkT [512, 2304] = WqkT.T @ xT, bf16 (bias on DVE);
    score lhsT/rhs slices use strided row x col access patterns.
  - v is re-laid per band chunk in [token, channel] order (partitions must
    match the band token order of the score rows); v bias is folded into
    the out-projection bias on the host (softmax weights sum to 1).
  - per (band, head-pair g, chunk): scoresT via row-packed K=32 matmuls,
    exp on ScalarE (scale fused), binary window mask multiply on VectorE,
    PV+sums accumulate via col-packed matmuls (ones lhsT strips replicate
    each head's sum onto the 32 partitions under its pv rows). The first
    PV matmul opens the bank with start=True (no zeroing matmul).
  - on = pp * shifted recip(sums) -> bf16; out proj reads the per-group
    `on` tiles directly with zero-padded weight rows (no compaction),
    +residual from xT with the folded bias, 1x1 conv, bias+relu on
    ScalarE, bf16 band-major output (host un-permutes).
"""

import sys

for _p in ("/opt/trn_rl_repo",):
    if _p not in sys.path:
        sys.path.insert(0, _p)

import math

import ml_dtypes
import numpy as np

B, C, H, W = 4, 256, 48, 48
N = H * W
HEADS, HD, HALF = 8, 32, 8
NCORES = 8
ROWS_HALF = H // 2          # 24 query rows per core
NQ = ROWS_HALF * W          # 1152 queries per core
BAND_ROWS = 32              # k/v row band per core (24 + 8 halo)
QT = 384                    # queries per band tile (24 rows x 16 cols)
NB = 3                      # column bands

SCALE = 1.0 / math.sqrt(HD)

# per band: first ext col, ext width
BANDC = [(0, 24), (8, 32), (24, 24)]
# per band: key chunk row ranges (chunk keys = rows x ext cols <= 128):
# edge bands (ew=24) use 5-row chunks (120 keys), center (ew=32) 4-row
_CH_EDGE = [(5 * j, min(32, 5 * j + 5)) for j in range(7)]
_CH_CENTER = [(4 * j, 4 * j + 4) for j in range(8)]
CH = [_CH_EDGE, _CH_CENTER, _CH_EDGE]
# query row window per chunk: [r0-8, r1+8) clipped to [0, 24)
def _win(r0, r1):
    return (max(0, r0 - 8), min(24, r1 + 8))
# packed mask column offsets: per band, per chunk
_off = 0
MOFF = []
for _b in range(NB):
    row = []
    for (_r0, _r1) in CH[_b]:
        row.append(_off)
        _rlo, _rhi = _win(_r0, _r1)
        _off += (_rhi - _rlo) * 16
    MOFF.append(row)
MTOT = _off

bf16 = ml_dtypes.bfloat16

_PROG = None


def _build_program():
    import concourse.bass as bass
    import concourse.mybir as mybir
    import concourse.tile as tile
    from concourse import bacc

    f32 = mybir.dt.float32
    bft = mybir.dt.bfloat16
    AF = mybir.ActivationFunctionType
    OP = mybir.AluOpType

    nc = bacc.Bacc(
        "TRN2", target_bir_lowering=False, debug=False, num_devices=NCORES
    )

    def din(name, shape, dt=bft):
        return nc.dram_tensor(name, shape, dt, kind="ExternalInput").ap()

    xt_d = din("xT", [128, 2, 1536])
    wqk_d = din("wqk", [128, 2, 2 * C])
    wv_d = din("wv", [128, 2, C])
    wo_d = din("wopad", [128, 4 * C])
    wc_d = din("wc", [128, 2, C])
    bias_d = din("biases", [128, 8], f32)
    mask_d = din("masks", [128, MTOT])
    out_d = nc.dram_tensor("out", [C, NB, QT], bft, kind="ExternalOutput").ap()

    # SPMD trick: one program must serve both row-halves. The host ships
    # half-1 images VERTICALLY FLIPPED (attention is equivariant under a
    # row flip; the window test is |dh|<=8), so every core sees half-0
    # geometry: query rows [0, 24), key band rows [0, 32).

    with tile.TileContext(nc) as tc:
        import contextlib

        ctx = contextlib.ExitStack()
        with ctx:
            cpool = ctx.enter_context(tc.tile_pool(name="const", bufs=1))
            qkpool = ctx.enter_context(tc.tile_pool(name="qk", bufs=1))
            vpool = ctx.enter_context(tc.tile_pool(name="v", bufs=1))
            epool = ctx.enter_context(tc.tile_pool(name="exp", bufs=4))
            apool = ctx.enter_context(tc.tile_pool(name="attn", bufs=4))
            rpool = ctx.enter_context(tc.tile_pool(name="recip", bufs=3))
            opool = ctx.enter_context(tc.tile_pool(name="outb", bufs=3))
            psA = ctx.enter_context(
                tc.tile_pool(name="psA", bufs=2, space="PSUM")
            )
            psB = ctx.enter_context(
                tc.tile_pool(name="psB", bufs=2, space="PSUM")
            )

            # ---- constants / inputs to SBUF (issue order = need order) ----
            # x rows 32-48 are never read (q uses [0,1152), k/v the 32-row
            # band [0,1536)), so only 1536 tokens are shipped.
            NX = 1536
            wqk = cpool.tile([128, 2, 2 * C], bft, tag="wqk")
            nc.sync.dma_start(wqk[:], wqk_d[:])
            xt = cpool.tile([128, 2, NX], bft, tag="xt")
            # 4 pieces so the first qk matmuls start ~3us earlier
            for pc in range(4):
                nc.sync.dma_start(
                    xt[:, :, 384 * pc : 384 * pc + 384],
                    xt_d[:, :, 384 * pc : 384 * pc + 384],
                )
            bias = cpool.tile([128, 8], f32, tag="bias")
            nc.sync.dma_start(bias[:], bias_d[:])
            msk = cpool.tile([128, MTOT], bft, tag="msk")
            nc.sync.dma_start(msk[:, 0 : MOFF[1][0]], mask_d[:, 0 : MOFF[1][0]])
            wv = cpool.tile([128, 2, C], bft, tag="wv")
            nc.sync.dma_start(wv[:], wv_d[:])
            nc.sync.dma_start(msk[:, MOFF[1][0] : MTOT], mask_d[:, MOFF[1][0] : MTOT])
            wo = cpool.tile([128, 4 * C], bft, tag="wo")
            nc.sync.dma_start(wo[:], wo_d[:])
            wc = cpool.tile([128, 2, C], bft, tag="wc")
            nc.sync.dma_start(wc[:], wc_d[:])
            zrow = cpool.tile([1, 512], bft, tag="zrow")
            nc.vector.memset(zrow[:], 0.0)

            # PE p-state warm-up: ~2.5us of junk matmuls so the real
            # projections hit the 3us-continuous-busy full-speed state
            # right as their inputs land (the ramp otherwise doubles the
            # first ~3us of matmul time)
            for _w in range(6):
                wps = psA.tile([128, 1024], f32, tag="sc", name="sc")
                nc.tensor.matmul(
                    wps[:, 0:512],
                    lhsT=zrow[:, 0:128],
                    rhs=zrow[:, 0:512],
                    start=True,
                    stop=True,
                )

            # ---- phase 1: qk projection  qkT[512, N] bf16 ----
            # q needed for tokens [0, 1152) only; k for the band [0, 1536)
            qk = [qkpool.tile([128, 1536], bft, tag=f"qk{i}", name=f"qk{i}") for i in range(4)]
            NT_Q = [(0, 384), (384, 384), (768, 384)]
            NT_K = [(0, 512), (512, 512), (1024, 512)]

            def qk_proj(qc):
                for n0, nw in (NT_Q if qc < 2 else NT_K):
                    ps = psB.tile([128, 512], f32, tag="w", name="w", bufs=4)
                    for cc in range(2):
                        nc.tensor.matmul(
                            ps[:, :nw],
                            lhsT=wqk[:, cc, 128 * qc : 128 * qc + 128],
                            rhs=xt[:, cc, n0 : n0 + nw],
                            start=(cc == 0),
                            stop=(cc == 1),
                        )
                    if qc < 2:
                        # q bias on DVE (fused with the bf16 cast)
                        nc.vector.tensor_scalar_add(
                            qk[qc][:, n0 : n0 + nw], ps[:, :nw], bias[:, qc : qc + 1]
                        )
                    else:
                        # the K bias shifts every key's logit by a constant
                        # per query, which softmax cancels exactly — drop
                        # it; the cast runs on the (early-idle) Act engine
                        nc.scalar.copy(qk[qc][:, n0 : n0 + nw], ps[:, :nw])

            # q reordered into band-major token order (matmul RHS APs must
            # have a single free dimension, so the band slices have to be
            # contiguous): qkb[ti][:, 384b + (r*16+c)] = q token (r, 16b+c)
            qkb = [qkpool.tile([128, NB * QT], bft, tag=f"qkb{i}", name=f"qkb{i}") for i in range(2)]

            def q_reorder(ti):
                src = qk[ti][:, 0:NQ].rearrange(
                    "p (r b c) -> p b r c", b=NB, c=16
                )
                nc.vector.tensor_copy(
                    qkb[ti][:].rearrange("p (b q) -> p b q", b=NB), src
                )

            # k and x re-laid in band-ext token order (matmul weight APs
            # must be a single free dimension too): 32 rows x ew cols per
            # band, bands packed at KOFF
            KOFF = [0, 768, 1792]
            KTOT = 2560
            kb = [qkpool.tile([128, KTOT], bft, tag=f"kb{i}", name=f"kb{i}") for i in range(2)]
            xb = cpool.tile([128, 2, KTOT], bft, tag="xb")

            def k_reorder(ti, bb):
                c0, ew = BANDC[bb]
                src = qk[2 + ti][:, :].rearrange("p (r w) -> p r w", w=W)[
                    :, :, c0 : c0 + ew
                ]
                dst = kb[ti][:, KOFF[bb] : KOFF[bb] + 32 * ew].rearrange(
                    "p (r c) -> p r c", c=ew
                )
                nc.vector.tensor_copy(dst, src)

            def x_reorder(cc, bb):
                c0, ew = BANDC[bb]
                src = xt[:, cc, 0 : 32 * W].rearrange("p (r w) -> p r w", w=W)[
                    :, :, c0 : c0 + ew
                ]
                dst = xb[:, cc, KOFF[bb] : KOFF[bb] + 32 * ew].rearrange(
                    "p (r c) -> p r c", c=ew
                )
                nc.vector.tensor_copy(dst, src)

            # v band chunk tiles, token-major partitions in band order
            # (chunk rows x extW); layout per tile: head h cols [64h,
            # 64h+32) = v_h, [64h+32, 64h+64) = 1.0
            vt = [
                [vpool.tile([128, 8 * 64], bft, tag=f"v{bb}_{j}", name=f"v{bb}_{j}")
                 for j in range(len(CH[bb]))]
                for bb in range(NB)
            ]

            def v_proj(bb, j):
                c0, ew = BANDC[bb]
                r0, r1 = CH[bb][j]
                m = (r1 - r0) * ew
                ko = KOFF[bb] + r0 * ew
                ps = psB.tile([128, 512], f32, tag="w", name="w", bufs=4)
                for cc in range(2):
                    nc.tensor.matmul(
                        ps[0:m, :C],
                        lhsT=xb[:, cc, ko : ko + m],
                        rhs=wv[:, cc, :],
                        start=(cc == 0),
                        stop=(cc == 1),
                    )
                va = vt[bb][j][:].rearrange("p (h two v) -> p h two v", two=2, v=32)
                cp = nc.vector if (bb + j) % 2 else nc.scalar
                if cp is nc.vector:
                    nc.vector.tensor_copy(
                        va[0:m, :, 0, :],
                        ps[0:m, :C].rearrange("p (h v) -> p h v", v=32),
                    )
                else:
                    nc.scalar.copy(
                        va[0:m, :, 0, :],
                        ps[0:m, :C].rearrange("p (h v) -> p h v", v=32),
                    )
                nc.gpsimd.memset(va[0:m, :, 1, :], 1.0)

            # Only band-0 / heads-0-3 inputs are prepared up front so
            # attention starts early; the rest is emitted into the PE
            # slack between group-pairs (see the hook schedule below).
            qk_proj(0)
            qk_proj(2)
            q_reorder(0)
            k_reorder(0, 0)
            for cc in range(2):
                x_reorder(cc, 0)
            v_proj(0, 0)
            v_proj(0, 1)

            def L(*fns):
                def emit():
                    for f in fns:
                        f()
                return emit

            # ---- phase 3: attention, per column band ----
            # Head-pair groups are processed in interleaved PAIRS (two
            # independent chunk pipelines) so each engine always has a
            # ready instruction from the other stream — single-stream
            # chunk chains leave 300-600ns handoff bubbles per chunk.
            res = [cpool.tile([128, NB * QT], bft, tag=f"res{t}", name=f"res{t}") for t in range(2)]

            def emit_scores(bb, g, ck):
                c0, ew = BANDC[bb]
                r0, r1 = CH[bb][ck]
                m = (r1 - r0) * ew
                ko = KOFF[bb] + r0 * ew
                rlo, rhi = _win(r0, r1)
                a, b = 16 * rlo, 16 * rhi
                qw_ = b - a
                sc = psA.tile([128, 1024], f32, tag="sc", name="sc")
                for hh in range(2):
                    h = 2 * g + hh
                    ti, krow = h // 4, 32 * (h % 4)
                    nc.tensor.matmul(
                        sc[0:m, 512 * hh + a : 512 * hh + b],
                        lhsT=kb[ti][krow : krow + 32, ko : ko + m],
                        rhs=qkb[ti][krow : krow + 32, QT * bb + a : QT * bb + b],
                        start=True,
                        stop=True,
                        tile_position=(krow, 0),
                    )
                ex = epool.tile([128, 2 * QT], bft, tag="ex", name="ex")
                sc_v = sc[0:m].rearrange("p (h q) -> p h q", q=512)[:, :, a:b]
                ex_v = ex[0:m].rearrange("p (h q) -> p h q", q=QT)[:, :, a:b]
                nc.scalar.activation(ex_v, sc_v, AF.Exp, scale=SCALE)
                ma = apool.tile([128, 2 * QT], bft, tag="ma", name="ma")
                ma_v = ma[0:m].rearrange("p (h q) -> p h q", q=QT)[:, :, a:b]
                mk = msk[0:m, MOFF[bb][ck] : MOFF[bb][ck] + qw_]
                nc.vector.tensor_mul(
                    ma_v, ex_v, mk[:, None, :].broadcast_to([m, 2, qw_])
                )
                return ma

            def emit_pv(bb, g, ck, ma, pp):
                c0, ew = BANDC[bb]
                r0, r1 = CH[bb][ck]
                m = (r1 - r0) * ew
                rlo, rhi = _win(r0, r1)
                a, b = 16 * rlo, 16 * rhi
                vi = vt[bb][ck]
                for hh in range(2):
                    h = 2 * g + hh
                    nc.tensor.matmul(
                        pp[64 * hh : 64 * hh + 64, a:b],
                        lhsT=vi[0:m, 64 * h : 64 * h + 64],
                        rhs=ma[0:m, QT * hh + a : QT * hh + b],
                        start=False,
                        stop=(ck == len(CH[bb]) - 1 and hh == 1),
                        skip_group_check=True,
                        tile_position=(0, 64 * hh),
                    )

            def emit_chunk(bb, g, ck, pp):
                emit_pv(bb, g, ck, emit_scores(bb, g, ck), pp)

            def emit_normalize(bb, g, pp):
                # rows of pp: 0-31 pv_a, 32-63 sums_a, 64-95 pv_b, 96-127 sums_b
                rc = rpool.tile([128, QT], f32, tag="rc", name="rc")
                nc.vector.reciprocal(rc[:], pp[:, 0:QT])
                on = rpool.tile([128, QT], bft, tag=f"on{g}", name=f"on{g}", bufs=2)
                if bb == NB - 1:
                    # shift-free normalize on the latency-critical last
                    # band: 32-partition base-offset multiplies (legal for
                    # PSUM x SBUF operands); rows 32-63 are read by the
                    # out proj against zero weights, so memset them.
                    nc.gpsimd.memset(on[32:64, :], 0.0)
                    nc.vector.tensor_mul(on[0:32, :], pp[0:32, 0:QT], rc[32:64, :])
                    nc.vector.tensor_mul(on[64:96, :], pp[64:96, 0:QT], rc[96:128, :])
                else:
                    # shift recip(sums) down 32 partitions onto pv lanes
                    rcs = rpool.tile([128, QT], f32, tag="rcs", name="rcs")
                    nc.sync.dma_start(rcs[0:96, :], rc[32:128, :])
                    nc.vector.tensor_mul(on[0:96, :], pp[0:96, 0:QT], rcs[0:96, :])
                return on

            # ---- projections for a band's columns (emitted after the
            # NEXT band's first pair so the PE stream hides them; the on
            # tiles rotate on a 2-ring so they stay live) ----
            def make_proj(bb, ons):
                def emit():
                    n0 = QT * bb
                    for oc in range(2):
                        ps = psB.tile([128, 512], f32, tag="w", name="w", bufs=4)
                        for g in range(4):
                            # out proj reads on tiles directly: rows 0-31 /
                            # 64-95 = normalized pv of heads 2g / 2g+1;
                            # wopad rows 32-63 are zero.
                            nc.tensor.matmul(
                                ps[:, :QT],
                                lhsT=wo[0:96, C * g + 128 * oc : C * g + 128 * oc + 128],
                                rhs=ons[g][0:96, :],
                                start=(g == 0),
                                stop=(g == 3),
                            )
                        xv = xt[:, oc, 0:NQ].rearrange(
                            "p (r w) -> p r w", w=W
                        )[:, :, 16 * bb : 16 * bb + 16]
                        nc.vector.scalar_tensor_tensor(
                            res[oc][:, n0 : n0 + QT].rearrange(
                                "p (r w) -> p r w", w=16
                            ),
                            ps[:, :QT].rearrange("p (r w) -> p r w", w=16),
                            bias[:, 6 + oc : 7 + oc],
                            xv,
                            OP.add,
                            OP.add,
                        )
                    for oc in range(2):
                        ps = psB.tile([128, 512], f32, tag="w", name="w", bufs=4)
                        for cc in range(2):
                            nc.tensor.matmul(
                                ps[:, :QT],
                                lhsT=wc[:, cc, 128 * oc : 128 * oc + 128],
                                rhs=res[cc][:, n0 : n0 + QT],
                                start=(cc == 0),
                                stop=(cc == 1),
                            )
                        ob = opool.tile([128, QT], bft, tag="ob", name="ob")
                        nc.scalar.activation(
                            ob[:],
                            ps[:, :QT],
                            AF.Relu,
                            bias=bias[:, 4 + oc : 5 + oc],
                        )
                        nc.sync.dma_start(
                            out_d[128 * oc : 128 * oc + 128, bb, :], ob[:]
                        )
                return emit

            # hook schedule: prep work spread finely into the PE/DVE slack
            # while the current pair's exp/mask stream runs — coarse
            # injections cause local engine bursts that starve Act
            after_pair = {}
            mid_hooks = {
                # band 0 remaining v tiles
                (0, 0, 0): [L(lambda: v_proj(0, 2), lambda: v_proj(0, 3))],
                (0, 0, 1): [L(*[lambda j=j: v_proj(0, j) for j in range(4, len(CH[0]))])],
                # heads 4-7 prep, needed by (0, 2)
                (0, 0, 2): [lambda: qk_proj(1)],
                (0, 0, 3): [lambda: qk_proj(3)],
                (0, 0, 4): [L(lambda: q_reorder(1), lambda: k_reorder(1, 0))],
                # band 1 prep, needed by (1, 0)
                (0, 2, 1): [L(lambda: k_reorder(0, 1), lambda: k_reorder(1, 1))],
                (0, 2, 2): [L(lambda: x_reorder(0, 1), lambda: x_reorder(1, 1))],
                (0, 2, 3): [L(*[lambda j=j: v_proj(1, j) for j in (0, 1, 2)])],
                (0, 2, 4): [L(*[lambda j=j: v_proj(1, j) for j in (3, 4)])],
                (0, 2, 5): [L(*[lambda j=j: v_proj(1, j) for j in range(5, len(CH[1]))])],
                # band 2 prep, needed by (2, 0)
                (1, 2, 1): [L(lambda: k_reorder(0, 2), lambda: k_reorder(1, 2))],
                (1, 2, 2): [L(lambda: x_reorder(0, 2), lambda: x_reorder(1, 2))],
                (1, 2, 3): [L(*[lambda j=j: v_proj(2, j) for j in (0, 1, 2)])],
                (1, 2, 4): [L(*[lambda j=j: v_proj(2, j) for j in (3, 4)])],
                (1, 2, 5): [L(*[lambda j=j: v_proj(2, j) for j in range(5, len(CH[2]))])],
            }

            # For the last band the out-proj accumulation is split: g0/g1
            # matmuls are emitted as soon as pair 0 normalizes (hidden
            # under pair 1's chunk stream), so the tail only runs g2/g3.
            last_ps = {}

            def proj_start_last(bb, ons):
                for oc in range(2):
                    ps = psB.tile([128, 512], f32, tag="w", name="w", bufs=4)
                    for g in (0, 1):
                        nc.tensor.matmul(
                            ps[:, :QT],
                            lhsT=wo[0:96, C * g + 128 * oc : C * g + 128 * oc + 128],
                            rhs=ons[g][0:96, :],
                            start=(g == 0),
                            stop=False,
                            skip_group_check=True,
                        )
                    last_ps[oc] = ps

            def proj_finish_last(bb, ons):
                n0 = QT * bb
                for oc in range(2):
                    ps = last_ps[oc]
                    for g in (2, 3):
                        nc.tensor.matmul(
                            ps[:, :QT],
                            lhsT=wo[0:96, C * g + 128 * oc : C * g + 128 * oc + 128],
                            rhs=ons[g][0:96, :],
                            start=False,
                            stop=(g == 3),
                            skip_group_check=True,
                        )
                    xv = xt[:, oc, 0:NQ].rearrange("p (r w) -> p r w", w=W)[
                        :, :, 16 * bb : 16 * bb + 16
                    ]
                    nc.vector.scalar_tensor_tensor(
                        res[oc][:, n0 : n0 + QT].rearrange("p (r w) -> p r w", w=16),
                        ps[:, :QT].rearrange("p (r w) -> p r w", w=16),
                        bias[:, 6 + oc : 7 + oc],
                        xv,
                        OP.add,
                        OP.add,
                    )
                for oc in range(2):
                    ps = psB.tile([128, 512], f32, tag="w", name="w", bufs=4)
                    for cc in range(2):
                        nc.tensor.matmul(
                            ps[:, :QT],
                            lhsT=wc[:, cc, 128 * oc : 128 * oc + 128],
                            rhs=res[cc][:, n0 : n0 + QT],
                            start=(cc == 0),
                            stop=(cc == 1),
                        )
                    ob = opool.tile([128, QT], bft, tag="ob", name="ob")
                    nc.scalar.activation(
                        ob[:], ps[:, :QT], AF.Relu, bias=bias[:, 4 + oc : 5 + oc]
                    )
                    nc.sync.dma_start(
                        out_d[128 * oc : 128 * oc + 128, bb, :], ob[:]
                    )

            proj = {}
            for bb in range(NB):
                ons = []
                for gp in (0, 2):
                    # The first chunk's scores/exp/mask are emitted BEFORE
                    # the pp zeroing matmuls: the zero-matmuls wait for the
                    # previous pair's normalize to free a pp ring slot, and
                    # the in-order PE queue would stall the new pair's
                    # whole score stream behind them.
                    mas = {g: emit_scores(bb, g, 0) for g in (gp, gp + 1)}
                    # pair tiles pp: rows = [pv_h|sums_h|pv_h'|sums_h'] for
                    # heads (2g, 2g+1); zeroed via an explicit start=True
                    # matmul (start=False on a cleared bank does NOT
                    # reliably SET on first write).
                    pps = {}
                    for g in (gp, gp + 1):
                        pp = psB.tile([128, 512], f32, tag="w", name="w", bufs=4)
                        nc.tensor.matmul(
                            pp[:, 0:QT],
                            lhsT=zrow[:, 0:128],
                            rhs=zrow[:, 0:QT],
                            start=True,
                            stop=False,
                            skip_group_check=True,
                        )
                        pps[g] = pp
                    for g in (gp, gp + 1):
                        emit_pv(bb, g, 0, mas[g], pps[g])
                    for fn in mid_hooks.get((bb, gp, 0), []):
                        fn()
                    for ck in range(1, len(CH[bb])):
                        for g in (gp, gp + 1):
                            emit_chunk(bb, g, ck, pps[g])
                        for fn in mid_hooks.get((bb, gp, ck), []):
                            fn()
                    for g in (gp, gp + 1):
                        ons.append(emit_normalize(bb, g, pps[g]))
                    for fn in after_pair.get((bb, gp), []):
                        fn()
                    if gp == 0 and bb > 0:
                        proj[bb - 1]()
                    if bb == NB - 1 and gp == 0:
                        proj_start_last(bb, ons)
                if bb < NB - 1:
                    proj[bb] = make_proj(bb, ons)
            proj_finish_last(NB - 1, ons)

    nc.compile()
    return nc


def _get_program():
    global _PROG
    if _PROG is None:
        _PROG = _build_program()
    return _PROG


def _prep_core_inputs(core, x, in_proj_w, in_proj_b, out_w, out_b, conv_w, conv_b):
    b, half = core // 2, core % 2
    ximg = x[b].reshape(C, H, W)
    if half == 1:
        ximg = ximg[:, ::-1, :]  # row-flip: half-1 becomes half-0 geometry
    xt = ximg.reshape(2, 128, N).transpose(1, 0, 2)

    wqk = in_proj_w[: 2 * C].T.reshape(2, 128, 2 * C).transpose(1, 0, 2)
    wv = in_proj_w[2 * C :].T.reshape(2, 128, C).transpose(1, 0, 2)
    wc = conv_w.T.reshape(2, 128, C).transpose(1, 0, 2)
    woT = out_w.T  # [in_ch, out_ch]
    wopad = np.zeros((128, 4 * C), np.float32)
    for g in range(4):
        wopad[0:32, C * g : C * g + C] = woT[64 * g : 64 * g + 32]
        wopad[64:96, C * g : C * g + C] = woT[64 * g + 32 : 64 * g + 64]

    # v bias folds through attention (softmax weights sum to 1) into the
    # out-proj bias: res = o@WoT + (out_b + bv@WoT) + x
    hostbias = out_b + in_proj_b[2 * C :] @ woT
    biases = np.zeros((128, 8), np.float32)
    biases[:, 0:4] = in_proj_b[: 2 * C].reshape(4, 128).T
    biases[:, 4:6] = conv_b.reshape(2, 128).T
    biases[:, 6:8] = hostbias.reshape(2, 128).T

    return {
        "xT": np.ascontiguousarray(xt[:, :, 0:1536]).astype(bf16),
        "wqk": np.ascontiguousarray(wqk).astype(bf16),
        "wv": np.ascontiguousarray(wv).astype(bf16),
        "wopad": wopad.astype(bf16),
        "wc": np.ascontiguousarray(wc).astype(bf16),
        "biases": biases,
        "masks": _masks(),
    }


_MASK_CACHE = {}


def _masks() -> np.ndarray:
    """[128, MTOT] binary window masks, packed per (band, chunk).

    Keys of chunk (band, j): rows [4j, 4j+4) x ext cols [c0, c0+ew),
    partition index (r-4j)*ew + (c-c0). Queries: rows [rlo, rhi) x band
    cols [16b, 16b+16), packed column index (r-rlo)*16 + (c-16b).
    """
    if "m" in _MASK_CACHE:
        return _MASK_CACHE["m"]
    out = np.zeros((128, MTOT), np.float32)
    for bb in range(NB):
        c0, ew = BANDC[bb]
        for j, (r0, r1) in enumerate(CH[bb]):
            rlo, rhi = _win(r0, r1)
            nk = (r1 - r0) * ew
            kr = r0 + np.arange(nk) // ew
            kc = c0 + np.arange(nk) % ew
            qr = rlo + np.arange((rhi - rlo) * 16) // 16
            qc = 16 * bb + np.arange((rhi - rlo) * 16) % 16
            m = (np.abs(kr[:, None] - qr[None, :]) <= HALF) & (
                np.abs(kc[:, None] - qc[None, :]) <= HALF
            )
            off = MOFF[bb][j]
            out[0:nk, off : off + (rhi - rlo) * 16] = m
    res = out.astype(bf16)
    _MASK_CACHE["m"] = res
    return res


def kernel(**inputs):
    from concourse.bass_utils import run_bass_kernel_spmd

    args = {k: np.asarray(v) for k, v in inputs.items()}
    nc = _get_program()
    in_maps = [
        _prep_core_inputs(core, **args) for core in range(NCORES)
    ]
    res = run_bass_kernel_spmd(nc, in_maps, core_ids=list(range(NCORES)))
    out = np.zeros((B, C, H, W), np.float32)
    for core in range(NCORES):
        b, half = core // 2, core % 2
        o = res.results[core]["out"].astype(np.float32)
        # band-major [C, 3, 24*16] -> [C, 24, 48]
        o = o.reshape(C, NB, ROWS_HALF, 16).transpose(0, 2, 1, 3).reshape(
            C, ROWS_HALF, W
        )
        if half == 1:
            o = o[:, ::-1, :]  # undo the row flip
            out[b][:, ROWS_HALF:, :] = o
        else:
            out[b][:, :ROWS_HALF, :] = o
    return out


# revision 57
# speedup vs baseline: 1.0652x; 1.0075x over previous
"""Trainium2 Bass kernel for ChunkedLocalSelfAttention.

Module: x[B,C,H,W] -> qkv proj -> 8-head local-window attention (17x17
spatial window) -> out proj -> +residual -> 1x1 conv -> relu.
B,C,H,W = 4,256,48,48; N = 2304 tokens per image; head dim 32.

Sharding: 8 cores = 4 batch images x 2 query-row-halves (24 rows each).
Each core computes the full pipeline for its half-image: attention output
rows only depend on +-8 image rows, so cores need no communication; the
row halo is covered by computing k/v for a 32-row band.

On-core design (scores kept TRANSPOSED: keys on partitions, queries free).
Attention is blocked in COLUMN BANDS of 16: queries of band b are the 24
rows x 16 cols [16b, 16b+16); its keys live in ext cols (+-8 halo,
clipped) x 32 rows, chunked into 8 chunks of 4 rows (<=128 keys). This
streams ~29% fewer score columns than full-width 128-token chunks since
the column halo is 24-32 wide instead of 48.
  - qk projection: qkT [512, 2304] = WqkT.T @ xT, bf16 (bias on DVE);
    score lhsT/rhs slices use strided row x col access patterns.
  - v is re-laid per band chunk in [token, channel] order (partitions must
    match the band token order of the score rows); v bias is folded into
    the out-projection bias on the host (softmax weights sum to 1).
  - per (band, head-pair g, chunk): scoresT via row-packed K=32 matmuls,
    exp on ScalarE (scale fused), binary window mask multiply on VectorE,
    PV+sums accumulate via col-packed matmuls (ones lhsT strips replicate
    each head's sum onto the 32 partitions under its pv rows). The first
    PV matmul opens the bank with start=True (no zeroing matmul).
  - on = pp * shifted recip(sums) -> bf16; out proj reads the per-group
    `on` tiles directly with zero-padded weight rows (no compaction),
    +residual from xT with the folded bias, 1x1 conv, bias+relu on
    ScalarE, bf16 band-major output (host un-permutes).
"""

import sys

for _p in ("/opt/trn_rl_repo",):
    if _p not in sys.path:
        sys.path.insert(0, _p)

import math

import ml_dtypes
import numpy as np

B, C, H, W = 4, 256, 48, 48
N = H * W
HEADS, HD, HALF = 8, 32, 8
NCORES = 8
ROWS_HALF = H // 2          # 24 query rows per core
NQ = ROWS_HALF * W          # 1152 queries per core
BAND_ROWS = 32              # k/v row band per core (24 + 8 halo)
QT = 384                    # queries per band tile (24 rows x 16 cols)
NB = 3                      # column bands

SCALE = 1.0 / math.sqrt(HD)

# per band: first ext col, ext width
BANDC = [(0, 24), (8, 32), (24, 24)]
# per band: key chunk row ranges (chunk keys = rows x ext cols <= 128):
# edge bands (ew=24) use 5-row chunks (120 keys), center (ew=32) 4-row
_CH_EDGE = [(5 * j, min(32, 5 * j + 5)) for j in range(7)]
_CH_CENTER = [(4 * j, 4 * j + 4) for j in range(8)]
CH = [_CH_EDGE, _CH_CENTER, _CH_EDGE]
# query row window per chunk: [r0-8, r1+8) clipped to [0, 24)
def _win(r0, r1):
    return (max(0, r0 - 8), min(24, r1 + 8))
# packed mask column offsets: per band, per chunk
_off = 0
MOFF = []
for _b in range(NB):
    row = []
    for (_r0, _r1) in CH[_b]:
        row.append(_off)
        _rlo, _rhi = _win(_r0, _r1)
        _off += (_rhi - _rlo) * 16
    MOFF.append(row)
MTOT = _off

bf16 = ml_dtypes.bfloat16

_PROG = None


def _build_program():
    import concourse.bass as bass
    import concourse.mybir as mybir
    import concourse.tile as tile
    from concourse import bacc

    f32 = mybir.dt.float32
    bft = mybir.dt.bfloat16
    AF = mybir.ActivationFunctionType
    OP = mybir.AluOpType

    nc = bacc.Bacc(
        "TRN2", target_bir_lowering=False, debug=False, num_devices=NCORES
    )

    def din(name, shape, dt=bft):
        return nc.dram_tensor(name, shape, dt, kind="ExternalInput").ap()

    xt_d = din("xT", [128, 2, 1536])
    wqk_d = din("wqk", [128, 2, 2 * C])
    wv_d = din("wv", [128, 2, C])
    wo_d = din("wopad", [128, 4 * C])
    wc_d = din("wc", [128, 2, C])
    bias_d = din("biases", [128, 8], f32)
    mask_d = din("masks", [128, MTOT])
    out_d = nc.dram_tensor("out", [C, NB, QT], bft, kind="ExternalOutput").ap()

    # SPMD trick: one program must serve both row-halves. The host ships
    # half-1 images VERTICALLY FLIPPED (attention is equivariant under a
    # row flip; the window test is |dh|<=8), so every core sees half-0
    # geometry: query rows [0, 24), key band rows [0, 32).

    with tile.TileContext(nc) as tc:
        import contextlib

        ctx = contextlib.ExitStack()
        with ctx:
            cpool = ctx.enter_context(tc.tile_pool(name="const", bufs=1))
            qkpool = ctx.enter_context(tc.tile_pool(name="qk", bufs=1))
            vpool = ctx.enter_context(tc.tile_pool(name="v", bufs=1))
            epool = ctx.enter_context(tc.tile_pool(name="exp", bufs=4))
            apool = ctx.enter_context(tc.tile_pool(name="attn", bufs=4))
            rpool = ctx.enter_context(tc.tile_pool(name="recip", bufs=3))
            opool = ctx.enter_context(tc.tile_pool(name="outb", bufs=3))
            psA = ctx.enter_context(
                tc.tile_pool(name="psA", bufs=2, space="PSUM")
            )
            psB = ctx.enter_context(
                tc.tile_pool(name="psB", bufs=2, space="PSUM")
            )

            # ---- constants / inputs to SBUF (issue order = need order) ----
            # x rows 32-48 are never read (q uses [0,1152), k/v the 32-row
            # band [0,1536)), so only 1536 tokens are shipped.
            NX = 1536
            wqk = cpool.tile([128, 2, 2 * C], bft, tag="wqk")
            nc.sync.dma_start(wqk[:], wqk_d[:])
            xt = cpool.tile([128, 2, NX], bft, tag="xt")
            # 4 pieces so the first qk matmuls start ~3us earlier
            for pc in range(4):
                nc.sync.dma_start(
                    xt[:, :, 384 * pc : 384 * pc + 384],
                    xt_d[:, :, 384 * pc : 384 * pc + 384],
                )
            bias = cpool.tile([128, 8], f32, tag="bias")
            nc.sync.dma_start(bias[:], bias_d[:])
            msk = cpool.tile([128, MTOT], bft, tag="msk")
            nc.sync.dma_start(msk[:, 0 : MOFF[1][0]], mask_d[:, 0 : MOFF[1][0]])
            wv = cpool.tile([128, 2, C], bft, tag="wv")
            nc.sync.dma_start(wv[:], wv_d[:])
            nc.sync.dma_start(msk[:, MOFF[1][0] : MTOT], mask_d[:, MOFF[1][0] : MTOT])
            wo = cpool.tile([128, 4 * C], bft, tag="wo")
            nc.sync.dma_start(wo[:], wo_d[:])
            wc = cpool.tile([128, 2, C], bft, tag="wc")
            nc.sync.dma_start(wc[:], wc_d[:])
            zrow = cpool.tile([1, 512], bft, tag="zrow")
            nc.vector.memset(zrow[:], 0.0)

            # PE p-state warm-up: ~2.5us of junk matmuls so the real
            # projections hit the 3us-continuous-busy full-speed state
            # right as their inputs land (the ramp otherwise doubles the
            # first ~3us of matmul time)
            for _w in range(6):
                wps = psA.tile([128, 1024], f32, tag="sc", name="sc")
                nc.tensor.matmul(
                    wps[:, 0:512],
                    lhsT=zrow[:, 0:128],
                    rhs=zrow[:, 0:512],
                    start=True,
                    stop=True,
                )

            # ---- phase 1: qk projection  qkT[512, N] bf16 ----
            # q needed for tokens [0, 1152) only; k for the band [0, 1536)
            qk = [qkpool.tile([128, 1536], bft, tag=f"qk{i}", name=f"qk{i}") for i in range(4)]
            NT_Q = [(0, 384), (384, 384), (768, 384)]
            NT_K = [(0, 512), (512, 512), (1024, 512)]

            def qk_proj(qc):
                for n0, nw in (NT_Q if qc < 2 else NT_K):
                    ps = psB.tile([128, 512], f32, tag="w", name="w", bufs=4)
                    for cc in range(2):
                        nc.tensor.matmul(
                            ps[:, :nw],
                            lhsT=wqk[:, cc, 128 * qc : 128 * qc + 128],
                            rhs=xt[:, cc, n0 : n0 + nw],
                            start=(cc == 0),
                            stop=(cc == 1),
                        )
                    if qc < 2:
                        # q bias on DVE (fused with the bf16 cast)
                        nc.vector.tensor_scalar_add(
                            qk[qc][:, n0 : n0 + nw], ps[:, :nw], bias[:, qc : qc + 1]
                        )
                    else:
                        # the K bias shifts every key's logit by a constant
                        # per query, which softmax cancels exactly — drop
                        # it; the cast runs on the (early-idle) Act engine
                        nc.scalar.copy(qk[qc][:, n0 : n0 + nw], ps[:, :nw])

            # q reordered into band-major token order (matmul RHS APs must
            # have a single free dimension, so the band slices have to be
            # contiguous): qkb[ti][:, 384b + (r*16+c)] = q token (r, 16b+c)
            qkb = [qkpool.tile([128, NB * QT], bft, tag=f"qkb{i}", name=f"qkb{i}") for i in range(2)]

            def q_reorder(ti):
                src = qk[ti][:, 0:NQ].rearrange(
                    "p (r b c) -> p b r c", b=NB, c=16
                )
                nc.vector.tensor_copy(
                    qkb[ti][:].rearrange("p (b q) -> p b q", b=NB), src
                )

            # k and x re-laid in band-ext token order (matmul weight APs
            # must be a single free dimension too): 32 rows x ew cols per
            # band, bands packed at KOFF
            KOFF = [0, 768, 1792]
            KTOT = 2560
            kb = [qkpool.tile([128, KTOT], bft, tag=f"kb{i}", name=f"kb{i}") for i in range(2)]
            xb = cpool.tile([128, 2, KTOT], bft, tag="xb")

            def k_reorder(ti, bb):
                c0, ew = BANDC[bb]
                src = qk[2 + ti][:, :].rearrange("p (r w) -> p r w", w=W)[
                    :, :, c0 : c0 + ew
                ]
                dst = kb[ti][:, KOFF[bb] : KOFF[bb] + 32 * ew].rearrange(
                    "p (r c) -> p r c", c=ew
                )
                nc.vector.tensor_copy(dst, src)

            def x_reorder(cc, bb):
                c0, ew = BANDC[bb]
                src = xt[:, cc, 0 : 32 * W].rearrange("p (r w) -> p r w", w=W)[
                    :, :, c0 : c0 + ew
                ]
                dst = xb[:, cc, KOFF[bb] : KOFF[bb] + 32 * ew].rearrange(
                    "p (r c) -> p r c", c=ew
                )
                nc.vector.tensor_copy(dst, src)

            # v band chunk tiles, token-major partitions in band order
            # (chunk rows x extW); layout per tile: head h cols [64h,
            # 64h+32) = v_h, [64h+32, 64h+64) = 1.0
            vt = [
                [vpool.tile([128, 8 * 64], bft, tag=f"v{bb}_{j}", name=f"v{bb}_{j}")
                 for j in range(len(CH[bb]))]
                for bb in range(NB)
            ]

            def v_proj(bb, j):
                c0, ew = BANDC[bb]
                r0, r1 = CH[bb][j]
                m = (r1 - r0) * ew
                ko = KOFF[bb] + r0 * ew
                ps = psB.tile([128, 512], f32, tag="w", name="w", bufs=4)
                for cc in range(2):
                    nc.tensor.matmul(
                        ps[0:m, :C],
                        lhsT=xb[:, cc, ko : ko + m],
                        rhs=wv[:, cc, :],
                        start=(cc == 0),
                        stop=(cc == 1),
                    )
                va = vt[bb][j][:].rearrange("p (h two v) -> p h two v", two=2, v=32)
                cp = nc.vector
                if cp is nc.vector:
                    nc.vector.tensor_copy(
                        va[0:m, :, 0, :],
                        ps[0:m, :C].rearrange("p (h v) -> p h v", v=32),
                    )
                else:
                    nc.scalar.copy(
                        va[0:m, :, 0, :],
                        ps[0:m, :C].rearrange("p (h v) -> p h v", v=32),
                    )
                nc.gpsimd.memset(va[0:m, :, 1, :], 1.0)

            # Only band-0 / heads-0-3 inputs are prepared up front so
            # attention starts early; the rest is emitted into the PE
            # slack between group-pairs (see the hook schedule below).
            qk_proj(0)
            qk_proj(2)
            q_reorder(0)
            k_reorder(0, 0)
            for cc in range(2):
                x_reorder(cc, 0)
            v_proj(0, 0)
            v_proj(0, 1)

            def L(*fns):
                def emit():
                    for f in fns:
                        f()
                return emit

            # ---- phase 3: attention, per column band ----
            # Head-pair groups are processed in interleaved PAIRS (two
            # independent chunk pipelines) so each engine always has a
            # ready instruction from the other stream — single-stream
            # chunk chains leave 300-600ns handoff bubbles per chunk.
            res = [cpool.tile([128, NB * QT], bft, tag=f"res{t}", name=f"res{t}") for t in range(2)]

            def emit_scores(bb, g, ck):
                c0, ew = BANDC[bb]
                r0, r1 = CH[bb][ck]
                m = (r1 - r0) * ew
                ko = KOFF[bb] + r0 * ew
                rlo, rhi = _win(r0, r1)
                a, b = 16 * rlo, 16 * rhi
                qw_ = b - a
                sc = psA.tile([128, 1024], f32, tag="sc", name="sc")
                for hh in range(2):
                    h = 2 * g + hh
                    ti, krow = h // 4, 32 * (h % 4)
                    nc.tensor.matmul(
                        sc[0:m, 512 * hh + a : 512 * hh + b],
                        lhsT=kb[ti][krow : krow + 32, ko : ko + m],
                        rhs=qkb[ti][krow : krow + 32, QT * bb + a : QT * bb + b],
                        start=True,
                        stop=True,
                        tile_position=(krow, 0),
                    )
                ex = epool.tile([128, 2 * QT], bft, tag="ex", name="ex")
                sc_v = sc[0:m].rearrange("p (h q) -> p h q", q=512)[:, :, a:b]
                ex_v = ex[0:m].rearrange("p (h q) -> p h q", q=QT)[:, :, a:b]
                nc.scalar.activation(ex_v, sc_v, AF.Exp, scale=SCALE)
                ma = apool.tile([128, 2 * QT], bft, tag="ma", name="ma")
                ma_v = ma[0:m].rearrange("p (h q) -> p h q", q=QT)[:, :, a:b]
                mk = msk[0:m, MOFF[bb][ck] : MOFF[bb][ck] + qw_]
                nc.vector.tensor_mul(
                    ma_v, ex_v, mk[:, None, :].broadcast_to([m, 2, qw_])
                )
                return ma

            def emit_pv(bb, g, ck, ma, pp):
                c0, ew = BANDC[bb]
                r0, r1 = CH[bb][ck]
                m = (r1 - r0) * ew
                rlo, rhi = _win(r0, r1)
                a, b = 16 * rlo, 16 * rhi
                vi = vt[bb][ck]
                for hh in range(2):
                    h = 2 * g + hh
                    nc.tensor.matmul(
                        pp[64 * hh : 64 * hh + 64, a:b],
                        lhsT=vi[0:m, 64 * h : 64 * h + 64],
                        rhs=ma[0:m, QT * hh + a : QT * hh + b],
                        start=False,
                        stop=(ck == len(CH[bb]) - 1 and hh == 1),
                        skip_group_check=True,
                        tile_position=(0, 64 * hh),
                    )

            def emit_chunk(bb, g, ck, pp):
                emit_pv(bb, g, ck, emit_scores(bb, g, ck), pp)

            def emit_normalize(bb, g, pp):
                # rows of pp: 0-31 pv_a, 32-63 sums_a, 64-95 pv_b, 96-127 sums_b
                rc = rpool.tile([128, QT], f32, tag="rc", name="rc")
                nc.vector.reciprocal(rc[:], pp[:, 0:QT])
                on = rpool.tile([128, QT], bft, tag=f"on{g}", name=f"on{g}", bufs=2)
                if bb == NB - 1:
                    # shift-free normalize on the latency-critical last
                    # band: 32-partition base-offset multiplies (legal for
                    # PSUM x SBUF operands); rows 32-63 are read by the
                    # out proj against zero weights, so memset them.
                    nc.gpsimd.memset(on[32:64, :], 0.0)
                    nc.vector.tensor_mul(on[0:32, :], pp[0:32, 0:QT], rc[32:64, :])
                    nc.vector.tensor_mul(on[64:96, :], pp[64:96, 0:QT], rc[96:128, :])
                else:
                    # shift recip(sums) down 32 partitions onto pv lanes
                    rcs = rpool.tile([128, QT], f32, tag="rcs", name="rcs")
                    nc.sync.dma_start(rcs[0:96, :], rc[32:128, :])
                    nc.vector.tensor_mul(on[0:96, :], pp[0:96, 0:QT], rcs[0:96, :])
                return on

            # ---- projections for a band's columns (emitted after the
            # NEXT band's first pair so the PE stream hides them; the on
            # tiles rotate on a 2-ring so they stay live) ----
            def make_proj(bb, ons):
                def emit():
                    n0 = QT * bb
                    for oc in range(2):
                        ps = psB.tile([128, 512], f32, tag="w", name="w", bufs=4)
                        for g in range(4):
                            # out proj reads on tiles directly: rows 0-31 /
                            # 64-95 = normalized pv of heads 2g / 2g+1;
                            # wopad rows 32-63 are zero.
                            nc.tensor.matmul(
                                ps[:, :QT],
                                lhsT=wo[0:96, C * g + 128 * oc : C * g + 128 * oc + 128],
                                rhs=ons[g][0:96, :],
                                start=(g == 0),
                                stop=(g == 3),
                            )
                        xv = xt[:, oc, 0:NQ].rearrange(
                            "p (r w) -> p r w", w=W
                        )[:, :, 16 * bb : 16 * bb + 16]
                        nc.vector.scalar_tensor_tensor(
                            res[oc][:, n0 : n0 + QT].rearrange(
                                "p (r w) -> p r w", w=16
                            ),
                            ps[:, :QT].rearrange("p (r w) -> p r w", w=16),
                            bias[:, 6 + oc : 7 + oc],
                            xv,
                            OP.add,
                            OP.add,
                        )
                    for oc in range(2):
                        ps = psB.tile([128, 512], f32, tag="w", name="w", bufs=4)
                        for cc in range(2):
                            nc.tensor.matmul(
                                ps[:, :QT],
                                lhsT=wc[:, cc, 128 * oc : 128 * oc + 128],
                                rhs=res[cc][:, n0 : n0 + QT],
                                start=(cc == 0),
                                stop=(cc == 1),
                            )
                        ob = opool.tile([128, QT], bft, tag="ob", name="ob")
                        nc.scalar.activation(
                            ob[:],
                            ps[:, :QT],
                            AF.Relu,
                            bias=bias[:, 4 + oc : 5 + oc],
                        )
                        nc.sync.dma_start(
                            out_d[128 * oc : 128 * oc + 128, bb, :], ob[:]
                        )
                return emit

            # hook schedule: prep work spread finely into the PE/DVE slack
            # while the current pair's exp/mask stream runs — coarse
            # injections cause local engine bursts that starve Act
            after_pair = {}
            mid_hooks = {
                # band 0 remaining v tiles
                (0, 0, 0): [L(lambda: v_proj(0, 2), lambda: v_proj(0, 3))],
                (0, 0, 1): [L(*[lambda j=j: v_proj(0, j) for j in range(4, len(CH[0]))])],
                # heads 4-7 prep, needed by (0, 2)
                (0, 0, 2): [lambda: qk_proj(1)],
                (0, 0, 3): [lambda: qk_proj(3)],
                (0, 0, 4): [L(lambda: q_reorder(1), lambda: k_reorder(1, 0))],
                # band 1 prep, needed by (1, 0)
                (0, 2, 1): [L(lambda: k_reorder(0, 1), lambda: k_reorder(1, 1))],
                (0, 2, 2): [L(lambda: x_reorder(0, 1), lambda: x_reorder(1, 1))],
                (0, 2, 3): [L(*[lambda j=j: v_proj(1, j) for j in (0, 1, 2)])],
                (0, 2, 4): [L(*[lambda j=j: v_proj(1, j) for j in (3, 4)])],
                (0, 2, 5): [L(*[lambda j=j: v_proj(1, j) for j in range(5, len(CH[1]))])],
                # band 2 prep, needed by (2, 0)
                (1, 2, 1): [L(lambda: k_reorder(0, 2), lambda: k_reorder(1, 2))],
                (1, 2, 2): [L(lambda: x_reorder(0, 2), lambda: x_reorder(1, 2))],
                (1, 2, 3): [L(*[lambda j=j: v_proj(2, j) for j in (0, 1, 2)])],
                (1, 2, 4): [L(*[lambda j=j: v_proj(2, j) for j in (3, 4)])],
                (1, 2, 5): [L(*[lambda j=j: v_proj(2, j) for j in range(5, len(CH[2]))])],
            }

            # For the last band the out-proj accumulation is split: g0/g1
            # matmuls are emitted as soon as pair 0 normalizes (hidden
            # under pair 1's chunk stream), so the tail only runs g2/g3.
            last_ps = {}

            def proj_start_last(bb, ons):
                for oc in range(2):
                    ps = psB.tile([128, 512], f32, tag="w", name="w", bufs=4)
                    for g in (0, 1):
                        nc.tensor.matmul(
                            ps[:, :QT],
                            lhsT=wo[0:96, C * g + 128 * oc : C * g + 128 * oc + 128],
                            rhs=ons[g][0:96, :],
                            start=(g == 0),
                            stop=False,
                            skip_group_check=True,
                        )
                    last_ps[oc] = ps

            def proj_finish_last(bb, ons):
                n0 = QT * bb
                for oc in range(2):
                    ps = last_ps[oc]
                    for g in (2, 3):
                        nc.tensor.matmul(
                            ps[:, :QT],
                            lhsT=wo[0:96, C * g + 128 * oc : C * g + 128 * oc + 128],
                            rhs=ons[g][0:96, :],
                            start=False,
                            stop=(g == 3),
                            skip_group_check=True,
                        )
                    xv = xt[:, oc, 0:NQ].rearrange("p (r w) -> p r w", w=W)[
                        :, :, 16 * bb : 16 * bb + 16
                    ]
                    nc.vector.scalar_tensor_tensor(
                        res[oc][:, n0 : n0 + QT].rearrange("p (r w) -> p r w", w=16),
                        ps[:, :QT].rearrange("p (r w) -> p r w", w=16),
                        bias[:, 6 + oc : 7 + oc],
                        xv,
                        OP.add,
                        OP.add,
                    )
                for oc in range(2):
                    ps = psB.tile([128, 512], f32, tag="w", name="w", bufs=4)
                    for cc in range(2):
                        nc.tensor.matmul(
                            ps[:, :QT],
                            lhsT=wc[:, cc, 128 * oc : 128 * oc + 128],
                            rhs=res[cc][:, n0 : n0 + QT],
                            start=(cc == 0),
                            stop=(cc == 1),
                        )
                    ob = opool.tile([128, QT], bft, tag="ob", name="ob")
                    nc.scalar.activation(
                        ob[:], ps[:, :QT], AF.Relu, bias=bias[:, 4 + oc : 5 + oc]
                    )
                    nc.sync.dma_start(
                        out_d[128 * oc : 128 * oc + 128, bb, :], ob[:]
                    )

            proj = {}
            for bb in range(NB):
                ons = []
                for gp in (0, 2):
                    # The first chunk's scores/exp/mask are emitted BEFORE
                    # the pp zeroing matmuls: the zero-matmuls wait for the
                    # previous pair's normalize to free a pp ring slot, and
                    # the in-order PE queue would stall the new pair's
                    # whole score stream behind them.
                    mas = {g: emit_scores(bb, g, 0) for g in (gp, gp + 1)}
                    # pair tiles pp: rows = [pv_h|sums_h|pv_h'|sums_h'] for
                    # heads (2g, 2g+1); zeroed via an explicit start=True
                    # matmul (start=False on a cleared bank does NOT
                    # reliably SET on first write).
                    pps = {}
                    for g in (gp, gp + 1):
                        pp = psB.tile([128, 512], f32, tag="w", name="w", bufs=4)
                        nc.tensor.matmul(
                            pp[:, 0:QT],
                            lhsT=zrow[:, 0:128],
                            rhs=zrow[:, 0:QT],
                            start=True,
                            stop=False,
                            skip_group_check=True,
                        )
                        pps[g] = pp
                    for g in (gp, gp + 1):
                        emit_pv(bb, g, 0, mas[g], pps[g])
                    for fn in mid_hooks.get((bb, gp, 0), []):
                        fn()
                    for ck in range(1, len(CH[bb])):
                        for g in (gp, gp + 1):
                            emit_chunk(bb, g, ck, pps[g])
                        for fn in mid_hooks.get((bb, gp, ck), []):
                            fn()
                    for g in (gp, gp + 1):
                        ons.append(emit_normalize(bb, g, pps[g]))
                    for fn in after_pair.get((bb, gp), []):
                        fn()
                    if gp == 0 and bb > 0:
                        proj[bb - 1]()
                    if bb == NB - 1 and gp == 0:
                        proj_start_last(bb, ons)
                if bb < NB - 1:
                    proj[bb] = make_proj(bb, ons)
            proj_finish_last(NB - 1, ons)

    nc.compile()
    return nc


def _get_program():
    global _PROG
    if _PROG is None:
        _PROG = _build_program()
    return _PROG


def _prep_core_inputs(core, x, in_proj_w, in_proj_b, out_w, out_b, conv_w, conv_b):
    b, half = core // 2, core % 2
    ximg = x[b].reshape(C, H, W)
    if half == 1:
        ximg = ximg[:, ::-1, :]  # row-flip: half-1 becomes half-0 geometry
    xt = ximg.reshape(2, 128, N).transpose(1, 0, 2)

    wqk = in_proj_w[: 2 * C].T.reshape(2, 128, 2 * C).transpose(1, 0, 2)
    wv = in_proj_w[2 * C :].T.reshape(2, 128, C).transpose(1, 0, 2)
    wc = conv_w.T.reshape(2, 128, C).transpose(1, 0, 2)
    woT = out_w.T  # [in_ch, out_ch]
    wopad = np.zeros((128, 4 * C), np.float32)
    for g in range(4):
        wopad[0:32, C * g : C * g + C] = woT[64 * g : 64 * g + 32]
        wopad[64:96, C * g : C * g + C] = woT[64 * g + 32 : 64 * g + 64]

    # v bias folds through attention (softmax weights sum to 1) into the
    # out-proj bias: res = o@WoT + (out_b + bv@WoT) + x
    hostbias = out_b + in_proj_b[2 * C :] @ woT
    biases = np.zeros((128, 8), np.float32)
    biases[:, 0:4] = in_proj_b[: 2 * C].reshape(4, 128).T
    biases[:, 4:6] = conv_b.reshape(2, 128).T
    biases[:, 6:8] = hostbias.reshape(2, 128).T

    return {
        "xT": np.ascontiguousarray(xt[:, :, 0:1536]).astype(bf16),
        "wqk": np.ascontiguousarray(wqk).astype(bf16),
        "wv": np.ascontiguousarray(wv).astype(bf16),
        "wopad": wopad.astype(bf16),
        "wc": np.ascontiguousarray(wc).astype(bf16),
        "biases": biases,
        "masks": _masks(),
    }


_MASK_CACHE = {}


def _masks() -> np.ndarray:
    """[128, MTOT] binary window masks, packed per (band, chunk).

    Keys of chunk (band, j): rows [4j, 4j+4) x ext cols [c0, c0+ew),
    partition index (r-4j)*ew + (c-c0). Queries: rows [rlo, rhi) x band
    cols [16b, 16b+16), packed column index (r-rlo)*16 + (c-16b).
    """
    if "m" in _MASK_CACHE:
        return _MASK_CACHE["m"]
    out = np.zeros((128, MTOT), np.float32)
    for bb in range(NB):
        c0, ew = BANDC[bb]
        for j, (r0, r1) in enumerate(CH[bb]):
            rlo, rhi = _win(r0, r1)
            nk = (r1 - r0) * ew
            kr = r0 + np.arange(nk) // ew
            kc = c0 + np.arange(nk) % ew
            qr = rlo + np.arange((rhi - rlo) * 16) // 16
            qc = 16 * bb + np.arange((rhi - rlo) * 16) % 16
            m = (np.abs(kr[:, None] - qr[None, :]) <= HALF) & (
                np.abs(kc[:, None] - qc[None, :]) <= HALF
            )
            off = MOFF[bb][j]
            out[0:nk, off : off + (rhi - rlo) * 16] = m
    res = out.astype(bf16)
    _MASK_CACHE["m"] = res
    return res


def kernel(**inputs):
    from concourse.bass_utils import run_bass_kernel_spmd

    args = {k: np.asarray(v) for k, v in inputs.items()}
    nc = _get_program()
    in_maps = [
        _prep_core_inputs(core, **args) for core in range(NCORES)
    ]
    res = run_bass_kernel_spmd(nc, in_maps, core_ids=list(range(NCORES)))
    out = np.zeros((B, C, H, W), np.float32)
    for core in range(NCORES):
        b, half = core // 2, core % 2
        o = res.results[core]["out"].astype(np.float32)
        # band-major [C, 3, 24*16] -> [C, 24, 48]
        o = o.reshape(C, NB, ROWS_HALF, 16).transpose(0, 2, 1, 3).reshape(
            C, ROWS_HALF, W
        )
        if half == 1:
            o = o[:, ::-1, :]  # undo the row flip
            out[b][:, ROWS_HALF:, :] = o
        else:
            out[b][:, :ROWS_HALF, :] = o
    return out
